# revision 16
# baseline (speedup 1.0000x reference)
"""Trainium2 Bass kernel for nn_RadialModel — triple-banded table version.

Gather cost on TRN2 SWDGE is ~9.8ns per descriptor regardless of batching,
so the lever is descriptor COUNT. Table cells hold THREE adjacent grid rows
(triple-banding): each point needs 2 gathers (rows fy+1, fy+4, bands of 3)
of 576B instead of 6 of 192B -> one third the descriptors.

The banded staging is built without cross-partition moves: DFT stage 2 runs
THREE times per v-tile with the stationary A-slice shifted by 0/1/2 columns,
directly yielding G[v+s, u] on partition v. Stage 1 (per-coil BT) is hoisted
before the vt loop; staging is one 6MB tile stored per v-tile.

A matrices carry two extra wrap columns (514) so shifted passes at vt=3
get A[:, 512..513] = A[:, 0..1] (grid rows 0..1).
"""
import numpy as np
import ml_dtypes

import concourse.bass as bass
import concourse.bacc as bacc
import concourse.mybir as mybir
import concourse.tile as tile
from concourse.bass_utils import run_bass_kernel_spmd
from concourse.masks import make_identity

F32 = mybir.dt.float32
I32 = mybir.dt.int32
AX = mybir.AxisListType
OP = mybir.AluOpType

IM = 256
G = 512
AV = 514           # A columns incl. wrap cols
J = 6
ALPHA = 2.34 * J
TWO_PI = 2.0 * np.pi
PAD = 517          # table cells per row: 512 + 2 left + 3 right halo
NT, NC, K = 8, 8, 16384
CELL = NC * 2      # 16 values per (row, col, b2) slot
BB = 3             # rows per band cell
TAU = 2            # bands per point
CW = BB * CELL     # 48 elems per table cell
NTILE = 16
GRP = 8
DEG = 8


def _host_consts():
    f = (np.arange(IM) - IM // 2) / G
    z = (np.pi * J * f) ** 2 - ALPHA ** 2
    s = np.sqrt(np.abs(z))
    val = np.where(z < 0, np.sinh(s) / np.maximum(s, 1e-12), np.sinc(s / np.pi))
    ftkb = (J / np.i0(ALPHA)) * val
    scal = 1.0 / ftkb
    u = np.arange(G)[:, None].astype(np.float64)
    xp = np.arange(IM)[None, :].astype(np.float64)
    A = np.exp(1j * np.pi * u / 2 - 2j * np.pi * u * xp / G) * scal[None, :] / np.sqrt(G)
    AT = A.T                                    # [256, 512]
    ATx = np.concatenate([AT, AT[:, 0:2]], axis=1)   # wrap cols
    art = np.ascontiguousarray(ATx.real.astype(ml_dtypes.bfloat16))
    ait = np.ascontiguousarray(ATx.imag.astype(ml_dtypes.bfloat16))
    aitn = np.ascontiguousarray((-ATx.imag).astype(ml_dtypes.bfloat16))
    n = 512
    x = (1 - np.cos(np.pi * (np.arange(n) + 0.5) / n)) / 2
    w = np.i0(ALPHA * np.sqrt(x)) / np.i0(ALPHA)
    V = np.vander(x, DEG + 1, increasing=True)
    c, *_ = np.linalg.lstsq(V, w, rcond=None)
    return art, ait, aitn, c.astype(np.float64)


_ART, _AIT, _AITN, _CHEB = _host_consts()


def build_bass():
    nc = bacc.Bacc()
    BF16 = mybir.dt.bfloat16

    x_in = nc.declare_dram_parameter("x", [2, IM, IM], F32, isOutput=False)
    k_in = nc.declare_dram_parameter("kk", [2, K], F32, isOutput=False)
    c_in = nc.declare_dram_parameter("coil", [NC, 2, IM, IM], F32, isOutput=False)
    w_in = nc.declare_dram_parameter("wr", [128, NTILE * 128], F32, isOutput=False)
    art_in = nc.declare_dram_parameter("art", [IM, AV], BF16, isOutput=False)
    ait_in = nc.declare_dram_parameter("ait", [IM, AV], BF16, isOutput=False)
    aitn_in = nc.declare_dram_parameter("aitn", [IM, AV], BF16, isOutput=False)
    y_out = nc.declare_dram_parameter("yr", [128, NTILE * 128], F32, isOutput=True)

    T_dram = nc.dram_tensor("T0", [G, PAD * CW], BF16)

    CH = _CHEB
    with tile.TileContext(nc) as tc:
        with (
            tc.tile_pool(name="const", bufs=1) as constp,
            tc.tile_pool(name="work", bufs=1) as workp,
            tc.tile_pool(name="ctile", bufs=2) as coilp,
            tc.tile_pool(name="mtile", bufs=4) as mp,
            tc.tile_pool(name="bt", bufs=1) as btp,
            tc.tile_pool(name="stg", bufs=1) as stgp,
            tc.tile_pool(name="patch", bufs=3) as patchp,
            tc.tile_pool(name="w36", bufs=2) as w36p,
            tc.tile_pool(name="wp", bufs=2) as wpp,
            tc.tile_pool(name="ps1", bufs=2, space="PSUM") as ps1,
            tc.tile_pool(name="ps2", bufs=3, space="PSUM") as ps2,
        ):
            # ---------------- constants ----------------
            ident = constp.tile([128, 128], F32, tag="ident")
            make_identity(nc, ident[:])
            art = []
            for name, src in (("art", art_in), ("ait", ait_in), ("aitn", aitn_in)):
                ts_ = []
                for xt in range(2):
                    t_ = constp.tile([128, AV], BF16, tag=f"{name}{xt}")
                    nc.sync.dma_start(out=t_[:], in_=src[xt * 128:(xt + 1) * 128, :])
                    ts_.append(t_)
                art.append(ts_)
            artT, aitT, aitnT = art

            offs = constp.tile([128, J], F32, tag="offs")
            offst = constp.tile([128, TAU], F32, tag="offst")
            for a in range(J):
                nc.vector.memset(offs[:, a:a + 1], float(3 - (a + 1)))
            for tt in range(TAU):
                nc.vector.memset(offst[:, tt:tt + 1], float(1 + BB * tt))

            # ---------------- k -> [p, c] transpose ----------------
            kg = workp.tile([128, 256], F32, tag="kg")
            for d in range(2):
                kt_in = workp.tile([128, 128], F32, tag="ktin")
                nc.sync.dma_start(
                    out=kt_in[:], in_=k_in[d].rearrange("(c p) -> c p", p=128)
                )
                ktp = ps2.tile([128, 2 * G], F32, tag="psb2")
                nc.tensor.transpose(ktp[:, 0:128], kt_in[:], ident[:])
                nc.scalar.copy(out=kg[:, d * 128:(d + 1) * 128], in_=ktp[:, 0:128])

            # ---------------- w load + sqrt ----------------
            wsq = workp.tile([128, NTILE * 128], F32, tag="wsq")
            nc.sync.dma_start(out=wsq[:], in_=w_in[:])
            nc.scalar.activation(
                out=wsq[:], in_=wsq[:],
                func=mybir.ActivationFunctionType.Sqrt,
            )

            # ---------------- index & weight math (DVE) ----------------
            gx0 = workp.tile([128, 256], F32, tag="gx0")
            nc.vector.tensor_scalar_mul(gx0[:], kg[:], float(G / TWO_PI))
            msk = workp.tile([128, 256], F32, tag="msk")
            nc.vector.tensor_scalar(
                out=msk[:], in0=gx0[:], scalar1=0.0, scalar2=None, op0=OP.is_lt
            )
            gxy = workp.tile([128, 256], F32, tag="gxy")
            nc.vector.scalar_tensor_tensor(
                out=gxy[:], in0=msk[:], scalar=float(G), in1=gx0[:],
                op0=OP.mult, op1=OP.add,
            )
            gm3 = workp.tile([128, 256], F32, tag="gm3")
            nc.vector.tensor_scalar(
                out=gm3[:], in0=gxy[:], scalar1=3.0, scalar2=None, op0=OP.subtract
            )
            fl = workp.tile([128, 256], F32, tag="fl")
            nc.vector.tensor_scalar(
                out=fl[:], in0=gm3[:],
                scalar1=-0.498046875, scalar2=12582912.0,
                op0=OP.add, op1=OP.add,
            )
            nc.vector.tensor_scalar(
                out=fl[:], in0=fl[:], scalar1=12582912.0, scalar2=None,
                op0=OP.subtract,
            )
            rr = workp.tile([128, 256], F32, tag="rr")
            nc.vector.tensor_sub(rr[:], gm3[:], fl[:])

            # 6-tap weights both dims (as baseline): acc[p, (d, c, a)]
            ut = workp.tile([128, 256 * J], F32, tag="ut")
            ut3 = ut[:].rearrange("p (dc a) -> p dc a", a=J)
            nc.vector.tensor_tensor(
                out=ut3,
                in0=rr[:].unsqueeze(2).broadcast_to([128, 256, J]),
                in1=offs[:].unsqueeze(1).broadcast_to([128, 256, J]),
                op=OP.add,
            )
            tsq = workp.tile([128, 256 * J], F32, tag="tsq")
            nc.vector.tensor_mul(tsq[:], ut[:], ut[:])
            nc.vector.tensor_scalar(
                out=tsq[:], in0=tsq[:], scalar1=float(-1.0 / 9.0), scalar2=1.0,
                op0=OP.mult, op1=OP.add,
            )
            nc.vector.tensor_scalar_max(tsq[:], tsq[:], 0.0)
            acc = workp.tile([128, 256 * J], F32, tag="acc")
            nc.vector.tensor_scalar(
                out=acc[:], in0=tsq[:], scalar1=float(CH[DEG]),
                scalar2=float(CH[DEG - 1]), op0=OP.mult, op1=OP.add,
            )
            for d in range(DEG - 2, -1, -1):
                nc.vector.tensor_mul(acc[:], acc[:], tsq[:])
                nc.vector.tensor_scalar_add(acc[:], acc[:], float(CH[d]))

            # band rows: rb[p,c,tau] = (fl_y + 1 + 2*tau) mod 512
            rb = workp.tile([128, 128 * TAU], F32, tag="rb")
            rb3 = rb[:].rearrange("p (c t) -> p c t", t=TAU)
            nc.vector.tensor_tensor(
                out=rb3,
                in0=fl[:, 128:256].unsqueeze(2).broadcast_to([128, 128, TAU]),
                in1=offst[:].unsqueeze(1).broadcast_to([128, 128, TAU]),
                op=OP.add,
            )
            mb = workp.tile([128, 128 * TAU], F32, tag="mb")
            nc.vector.tensor_scalar(
                out=mb[:], in0=rb[:], scalar1=0.0, scalar2=None, op0=OP.is_lt
            )
            nc.vector.scalar_tensor_tensor(
                out=rb[:], in0=mb[:], scalar=512.0, in1=rb[:],
                op0=OP.mult, op1=OP.add,
            )
            nc.vector.tensor_scalar(
                out=mb[:], in0=rb[:], scalar1=511.5, scalar2=None, op0=OP.is_gt
            )
            nc.vector.scalar_tensor_tensor(
                out=rb[:], in0=mb[:], scalar=-512.0, in1=rb[:],
                op0=OP.mult, op1=OP.add,
            )
            # idx = rb*(PAD*2) + (fl_x + 3)*2   (flat cells of 16 elems)
            flx3 = workp.tile([128, 128], F32, tag="flx3")
            nc.vector.tensor_scalar(
                out=flx3[:], in0=fl[:, 0:128], scalar1=3.0, scalar2=float(BB),
                op0=OP.add, op1=OP.mult,
            )
            idxf = workp.tile([128, 128 * TAU], F32, tag="idxf")
            nc.vector.tensor_scalar_mul(idxf[:], rb[:], float(PAD * BB))
            idxf3 = idxf[:].rearrange("p (c t) -> p c t", t=TAU)
            nc.vector.tensor_tensor(
                out=idxf3, in0=idxf3,
                in1=flx3[:].unsqueeze(2).broadcast_to([128, 128, TAU]),
                op=OP.add,
            )
            idx32 = workp.tile([128, 128 * TAU], I32, tag="idx32")
            nc.vector.tensor_copy(out=idx32[:], in_=idxf[:])

            # ---------------- res buffer ----------------
            res = workp.tile([128, NTILE * 128], F32, tag="res")

            # x image tiles
            xts = []
            for xt in range(2):
                xt_t = workp.tile([128, 2 * IM], F32, tag=f"xt{xt}")
                nc.sync.dma_start(
                    out=xt_t[:],
                    in_=x_in[:, xt * 128:(xt + 1) * 128, :]
                    .rearrange("ri x y -> x ri y"),
                )
                xts.append(xt_t)

            # ---- phase A1: all coils stage 1 -> persistent bt tiles ----
            bt = {}
            for c in range(NC):
                mt = []
                for xt in range(2):
                    ct = coilp.tile([128, 2 * IM], F32, tag="ct")
                    nc.sync.dma_start(
                        out=ct[:],
                        in_=c_in[c, :, xt * 128:(xt + 1) * 128, :]
                        .rearrange("ri x y -> x ri y"),
                    )
                    xt_t = xts[xt]
                    m = mp.tile([128, 2 * IM], BF16, tag="m")
                    xr, xi = xt_t[:, 0:IM], xt_t[:, IM:2 * IM]
                    cr, ci = ct[:, 0:IM], ct[:, IM:2 * IM]
                    mr, mi = m[:, 0:IM], m[:, IM:2 * IM]
                    t1 = mp.tile([128, IM], F32, tag="cm1")
                    t2 = mp.tile([128, IM], F32, tag="cm2")
                    nc.vector.tensor_mul(t1[:], xr, cr)
                    nc.vector.tensor_mul(t2[:], xi, ci)
                    nc.vector.tensor_sub(mr, t1[:], t2[:])
                    nc.vector.tensor_mul(t1[:], xr, ci)
                    nc.vector.tensor_mul(t2[:], xi, cr)
                    nc.vector.tensor_add(mi, t1[:], t2[:])
                    mt.append(m)
                for yt in range(2):
                    pr = ps1.tile([128, G], F32, tag="psa")
                    pi = ps1.tile([128, G], F32, tag="psa")
                    for xt in range(2):
                        mrb = mt[xt][:, yt * 128:yt * 128 + 128]
                        mib = mt[xt][:, IM + yt * 128:IM + yt * 128 + 128]
                        st = xt == 0
                        sp = xt == 1
                        a0 = artT[xt][:, 0:G]
                        a1 = aitT[xt][:, 0:G]
                        a1n = aitnT[xt][:, 0:G]
                        nc.tensor.matmul(pr[:], mrb, a0, start=st, stop=False)
                        nc.tensor.matmul(pi[:], mrb, a1, start=st, stop=False)
                        nc.tensor.matmul(pr[:], mib, a1n, start=False, stop=sp)
                        nc.tensor.matmul(pi[:], mib, a0, start=False, stop=sp)
                    btr = btp.tile([128, G], BF16, tag=f"bt{c}_{yt}_r")
                    bti = btp.tile([128, G], BF16, tag=f"bt{c}_{yt}_i")
                    nc.scalar.copy(out=btr[:], in_=pr[:])
                    nc.vector.tensor_copy(out=bti[:], in_=pi[:])
                    bt[(c, 0, yt)] = btr
                    bt[(c, 1, yt)] = bti

            # ---- phase A2: per v-tile, 2 shifted stage-2 passes ----
            drain_engs = [nc.scalar, nc.vector]
            t_stores = []
            di = 0
            for vt in range(4):
                stg = stgp.tile([128, G * CW], BF16, tag="stg2")
                stg3 = stg[:].rearrange("p (u b e) -> p u b e", b=BB, e=CELL)
                for c in range(NC):
                    for s in range(BB):
                        pst = ps2.tile([128, 2 * G], F32, tag="psb2")
                        gr = pst[:, 0:G]
                        gi = pst[:, G:2 * G]
                        for yt in range(2):
                            av = artT[yt][:, vt * 128 + s:vt * 128 + s + 128]
                            aiv = aitT[yt][:, vt * 128 + s:vt * 128 + s + 128]
                            ainv = aitnT[yt][:, vt * 128 + s:vt * 128 + s + 128]
                            btr = bt[(c, 0, yt)]
                            bti = bt[(c, 1, yt)]
                            st = yt == 0
                            sp = yt == 1
                            nc.tensor.matmul(gr, av, btr[:], start=st, stop=False)
                            nc.tensor.matmul(gi, aiv, btr[:], start=st, stop=False)
                            nc.tensor.matmul(gr, ainv, bti[:], start=False, stop=sp)
                            nc.tensor.matmul(gi, av, bti[:], start=False, stop=sp)
                        e0 = drain_engs[di % 2]
                        di += 1
                        dst = bass.AP(
                            stg[:].tensor,
                            stg[:].offset + s * CELL + 2 * c,
                            [stg[:].ap[0], [CW, G], [1, 2]],
                        )
                        srcv = bass.AP(
                            pst[:].tensor, pst[:].offset,
                            [pst[:].ap[0], [1, G], [G, 2]],
                        )
                        if e0 is nc.scalar:
                            e0.copy(out=dst, in_=srcv)
                        else:
                            e0.tensor_copy(out=dst, in_=srcv)
                r0 = vt * 128
                # split the main store across two HWDGE engines: halves the
                # serial store latency on the phase-A -> gather critical path
                half = (G // 2) * CW
                t_stores.append(nc.sync.dma_start(
                    out=T_dram[r0:r0 + 128, 2 * CW:2 * CW + half],
                    in_=stg[:, 0:half],
                ))
                t_stores.append(nc.scalar.dma_start(
                    out=T_dram[r0:r0 + 128, 2 * CW + half:2 * CW + G * CW],
                    in_=stg[:, half:G * CW],
                ))
                t_stores.append(nc.sync.dma_start(
                    out=T_dram[r0:r0 + 128, 0:2 * CW],
                    in_=stg[:, 510 * CW:512 * CW],
                ))
                t_stores.append(nc.sync.dma_start(
                    out=T_dram[r0:r0 + 128, 514 * CW:517 * CW],
                    in_=stg[:, 0:3 * CW],
                ))

            # ======== gather + combine ========
            tab_flat = T_dram[:].rearrange("r (q e) -> (r q) e", e=CELL)
            all_gathers = []
            for t in range(NTILE):
                # w36[p, (g, tau, c, b2)] = wx[c] * wy[2*tau + b2], 3 builds
                w36 = w36p.tile([128, GRP * J * J], F32, tag="w36")
                for tt in range(TAU):
                    wxv = bass.AP(
                        acc[:].tensor, acc[:].offset + t * 48,
                        [acc[:].ap[0], [J, GRP], [1, J], [0, BB]],
                    )
                    wyv = bass.AP(
                        acc[:].tensor, acc[:].offset + 768 + t * 48 + BB * tt,
                        [acc[:].ap[0], [J, GRP], [0, J], [1, BB]],
                    )
                    w36v = bass.AP(
                        w36[:].tensor, w36[:].offset + tt * J * BB,
                        [w36[:].ap[0], [J * J, GRP], [BB, J], [1, BB]],
                    )
                    nc.vector.tensor_tensor(out=w36v, in0=wxv, in1=wyv, op=OP.mult)
                patch = patchp.tile([128, GRP * TAU * J * CW], BF16, tag="patch")
                for g in range(GRP):
                    for tt in range(TAU):
                        col = (t * GRP + g) * TAU + tt
                        gi_ = nc.gpsimd.indirect_dma_start(
                            out=patch[:, (g * TAU + tt) * J * CW:
                                      (g * TAU + tt + 1) * J * CW],
                            out_offset=None,
                            in_=tab_flat,
                            in_offset=bass.IndirectOffsetOnAxis(
                                ap=idx32[:, col:col + 1], axis=0
                            ),
                        )
                        all_gathers.append(gi_)
                # wp[p, (g, cr, (tau,c,b2))] = patch * w36
                wp = wpp.tile([128, GRP * J * J * CELL], BF16, tag="wpt")
                for tt in range(TAU):
                    pv = bass.AP(
                        patch[:].tensor, patch[:].offset + tt * J * CW,
                        [patch[:].ap[0],
                         [TAU * J * CW, GRP], [1, CELL], [CELL, J * BB]],
                    )
                    wv = bass.AP(
                        w36[:].tensor, w36[:].offset + tt * J * BB,
                        [w36[:].ap[0], [J * J, GRP], [0, CELL], [1, J * BB]],
                    )
                    ov = bass.AP(
                        wp[:].tensor, wp[:].offset + tt * J * BB,
                        [wp[:].ap[0],
                         [J * J * CELL, GRP], [J * J, CELL], [1, J * BB]],
                    )
                    nc.vector.tensor_tensor(out=ov, in0=pv, in1=wv, op=OP.mult)
                rv = bass.AP(
                    res[:].tensor, res[:].offset + t * 128,
                    [res[:].ap[0], [16, GRP], [1, CELL]],
                )
                wp3 = wp[:].rearrange("p (g cr ba) -> p g cr ba",
                                      cr=CELL, ba=J * J)
                nc.vector.tensor_reduce(out=rv, in_=wp3, axis=AX.X, op=OP.add)

            for gi_ in all_gathers:
                for si in t_stores:
                    tile.add_dep_helper(gi_.ins, si.ins, reason="T RAW")

            # ======== sqrt(w) scale + store ========
            nc.vector.tensor_mul(res[:], res[:], wsq[:])
            nc.sync.dma_start(out=y_out[:], in_=res[:])

    nc.compile()
    return nc


_NC_CACHE = None


def _get_nc():
    global _NC_CACHE
    if _NC_CACHE is None:
        _NC_CACHE = build_bass()
    return _NC_CACHE


def _shuffle_w(w_t):
    v = w_t.reshape(NC, 2, NTILE, GRP, 128)
    return np.ascontiguousarray(v.transpose(4, 2, 3, 0, 1).reshape(128, NTILE * 128))


def _unshuffle_y(yr):
    v = yr.reshape(128, NTILE, GRP, NC, 2)
    return np.ascontiguousarray(v.transpose(3, 4, 1, 2, 0).reshape(NC, 2, K))


def make_in_maps(x, k, coil_sensitivities, w):
    in_maps = []
    coil0 = np.ascontiguousarray(coil_sensitivities[0], dtype=np.float32)
    for t in range(NT):
        in_maps.append({
            "x": np.ascontiguousarray(x[t], dtype=np.float32),
            "kk": np.ascontiguousarray(k[t], dtype=np.float32),
            "coil": coil0,
            "wr": _shuffle_w(np.asarray(w[t], dtype=np.float32)),
            "art": _ART, "ait": _AIT, "aitn": _AITN,
        })
    return in_maps


def run(x, k, coil_sensitivities, w, trace=False, **spmd_kwargs):
    nc = _get_nc()
    in_maps = make_in_maps(x, k, coil_sensitivities, w)
    r = run_bass_kernel_spmd(nc, in_maps, list(range(NT)), trace=trace, **spmd_kwargs)
    y = np.stack([_unshuffle_y(r.results[t]["yr"]) for t in range(NT)], axis=0)
    return y.astype(np.float32), r


def kernel(x, k, coil_sensitivities, w):
    y, _ = run(x, k, coil_sensitivities, w, trace=False)
    return y
